# revision 19
# baseline (speedup 1.0000x reference)
"""ConvAConnect Trainium2 kernel.

Per-sample noisy conv: Z[b] = conv2d(X[b], W * Werr[b], VALID) + bias * Berr[b].

Strategy: data-parallel over batch across 8 NeuronCores (8 samples each).
Per core, the conv is lowered to 9 shifted matmuls (one per 3x3 tap)
accumulating in PSUM:
  out[(ho,wo), cout] += X[(ho+kh, wo+kw), cin] @ (W*Werr)[kh,kw,cin,cout]
with Cin=128 exactly the PE contraction dim. X is pre-transposed on the
host to [Cin, H*W] so both matmul operands have Cin on partitions and all
DMAs are contiguous. Matmuls run in float32r (fp32 operands, FP22
multiply, full PE rate at moving-dim >= 256), accumulate fp32 in PSUM.
Output chunks are 2 output rows (M=124 partitions, N=256) which are
directly contiguous NHWC rows in DRAM. The per-sample bias
(bias * Berr[b]) is added during the PSUM->SBUF move by the DVE.
"""

import numpy as np

B, H, Wd, CIN, COUT, KH, KW = 64, 64, 64, 128, 256, 3, 3
HO, WO = H - KH + 1, Wd - KW + 1  # 62, 62
NCORES = 8
S = B // NCORES  # samples per core
ROWS_PER_CHUNK = 2
NCHUNK = HO // ROWS_PER_CHUNK  # 31
M = ROWS_PER_CHUNK * WO  # 124

PAD = 64  # X tile free-dim pad: last chunk's kh=2/kw>0 taps read past H*W

TRACE = False  # set by test harness to capture an NTFF profile
LAST_RESULTS = None  # BassKernelResults of the most recent run (for profiling)

_prog_cache = None


def _build_program():
    import concourse.mybir as mybir
    from concourse import bacc
    from concourse.tile import TileContext

    f32 = mybir.dt.float32
    f32r = mybir.dt.float32r

    # Bacc (not plain Bass): its compile() runs generate_event_semaphores,
    # which splits multi-sem waits into EventSemaphore chains — walrus
    # codegen rejects instructions carrying more than ~2 sync waits.
    nc = bacc.Bacc()

    # X_t is declared float32r (same bytes as f32): walrus requires data
    # consumed by an f32r matmul to be produced as f32r along the whole chain.
    # The free dim carries a host-zeroed PAD so the last chunk's kh=2 taps
    # can read one full 128-wide stationary without going out of bounds.
    X_t = nc.declare_dram_parameter(
        "X_t", [S, CIN, H * Wd + PAD], f32r, isOutput=False
    )
    W_p = nc.declare_dram_parameter("W", [KH * KW * CIN, COUT], f32, isOutput=False)
    bias_p = nc.declare_dram_parameter("bias", [COUT], f32, isOutput=False)
    Werr_p = nc.declare_dram_parameter(
        "Werr", [S, KH * KW * CIN, COUT], f32, isOutput=False
    )
    Berr_p = nc.declare_dram_parameter("Berr", [S, COUT], f32, isOutput=False)
    # output rows are stored 64 wide (2 dead columns) so each chunk is one
    # full-partition DMA; the host strips the padding
    OUT = nc.declare_dram_parameter("OUT", [S, HO * Wd, COUT], f32, isOutput=True)

    TAPF = KH * KW * COUT  # 2304 free elems: tap t occupies cols [t*COUT, (t+1)*COUT)

    with TileContext(nc) as tc:
        with (
            tc.tile_pool(name="const", bufs=1) as cpool,
            tc.tile_pool(name="xp", bufs=2) as xpool,
            tc.tile_pool(name="wep", bufs=2) as wepool,
            tc.tile_pool(name="mwp", bufs=2) as mwpool,
            tc.tile_pool(name="bbp", bufs=2) as bbpool,
            tc.tile_pool(name="outp", bufs=6) as opool,
            tc.tile_pool(name="ps", bufs=8, space="PSUM") as pspool,
        ):
            # W taps, resident all run: [cin, (t cout)]
            W_sb = cpool.tile([CIN, TAPF], f32)
            nc.sync.dma_start(
                out=W_sb[:].rearrange("p (t j) -> p t j", j=COUT),
                in_=W_p[:].rearrange("(t p) j -> p t j", p=CIN),
            )
            # bias broadcast to all partitions: [128, COUT]
            bias_bc = cpool.tile([128, COUT], f32)
            nc.gpsimd.dma_start(out=bias_bc, in_=bias_p[:].partition_broadcast(128))

            for s in range(S):
                X_sb = xpool.tile([CIN, H * Wd + PAD], f32r)
                nc.sync.dma_start(out=X_sb, in_=X_t[s])

                Werr_sb = wepool.tile([CIN, TAPF], f32)
                nc.sync.dma_start(
                    out=Werr_sb[:].rearrange("p (t j) -> p t j", j=COUT),
                    in_=Werr_p[s].rearrange("(t p) j -> p t j", p=CIN),
                )
                memW = mwpool.tile([CIN, TAPF], f32r)
                nc.vector.tensor_mul(memW, W_sb, Werr_sb)

                berr_bc = bbpool.tile([128, COUT], f32)
                nc.gpsimd.dma_start(
                    out=berr_bc, in_=Berr_p[s].partition_broadcast(128)
                )
                membias = bbpool.tile([128, COUT], f32)
                nc.vector.tensor_mul(membias, bias_bc, berr_bc)

                # Each chunk covers 2 output rows as 128 PSUM partitions in
                # 64-wide row coordinates: partition m = (ho - 2c)*64 + wo,
                # wo in [0,64) with wo in {62,63} dead. The tap (kh,kw)
                # stationary is then the single contiguous X slab starting at
                # (2c+kh)*64 + kw — one free dim, as walrus requires.
                for c in range(NCHUNK):
                    ps = pspool.tile([128, COUT], f32)
                    mm = 0
                    for kh in range(KH):
                        for kw in range(KW):
                            t = kh * KW + kw
                            base = (ROWS_PER_CHUNK * c + kh) * Wd + kw
                            lhsT = X_sb[:, base : base + 128]
                            rhs = memW[:, t * COUT : (t + 1) * COUT]  # [128, 256]
                            nc.tensor.matmul(
                                ps[:],
                                lhsT,
                                rhs,
                                start=(mm == 0),
                                stop=(mm == KH * KW - 1),
                            )
                            mm += 1
                    o_sb = opool.tile([128, COUT], f32)
                    nc.vector.tensor_add(o_sb, ps, membias)
                    # out stores ride ACT's HWDGE so SP's queue clocks (wide
                    # X/Werr loads) and these narrow stores stay independent
                    nc.scalar.dma_start(
                        out=OUT[s, 128 * c : 128 * (c + 1), :], in_=o_sb
                    )

    nc.compile()
    return nc


def _get_program():
    global _prog_cache
    if _prog_cache is None:
        _prog_cache = _build_program()
    return _prog_cache


def kernel(X, W, bias, Werr, Berr):
    global LAST_RESULTS
    from concourse.bass_utils import run_bass_kernel_spmd

    X = np.asarray(X, dtype=np.float32)
    W = np.asarray(W, dtype=np.float32)
    bias = np.asarray(bias, dtype=np.float32)
    Werr = np.asarray(Werr, dtype=np.float32)
    Berr = np.asarray(Berr, dtype=np.float32)

    # host-side layout prep (part of sharding): Cin onto partitions, zero pad
    X_t = np.zeros((B, CIN, H * Wd + PAD), np.float32)
    X_t[:, :, : H * Wd] = X.transpose(0, 3, 1, 2).reshape(B, CIN, H * Wd)
    W2 = np.ascontiguousarray(W.reshape(KH * KW * CIN, COUT))
    Werr2 = np.ascontiguousarray(Werr.reshape(B, KH * KW * CIN, COUT))
    Berr2 = np.ascontiguousarray(Berr)

    nc = _get_program()
    in_maps = []
    for core in range(NCORES):
        sl = slice(core * S, (core + 1) * S)
        in_maps.append(
            {
                "X_t": X_t[sl],
                "W": W2,
                "bias": bias,
                "Werr": Werr2[sl],
                "Berr": Berr2[sl],
            }
        )

    res = run_bass_kernel_spmd(
        nc, in_maps, core_ids=list(range(NCORES)), trace=TRACE
    )
    LAST_RESULTS = res
    out = np.concatenate([r["OUT"] for r in res.results], axis=0)
    # rows are stored 64 wide on device; strip the 2 dead columns
    return np.ascontiguousarray(
        out.reshape(B, HO, Wd, COUT)[:, :, :WO, :]
    )


# revision 20
# speedup vs baseline: 1.0477x; 1.0477x over previous
"""ConvAConnect Trainium2 kernel.

Per-sample noisy conv: Z[b] = conv2d(X[b], W * Werr[b], VALID) + bias * Berr[b].

Strategy: data-parallel over batch across 8 NeuronCores (8 samples each).
Per core, the conv is lowered to 9 shifted matmuls (one per 3x3 tap)
accumulating in PSUM:
  out[(ho,wo), cout] += X[(ho+kh, wo+kw), cin] @ (W*Werr)[kh,kw,cin,cout]
with Cin=128 exactly the PE contraction dim. X is pre-transposed on the
host to [Cin, H*W] so both matmul operands have Cin on partitions and all
DMAs are contiguous. Matmuls run in float32r (fp32 operands, FP22
multiply, full PE rate at moving-dim >= 256), accumulate fp32 in PSUM.
Output chunks are 2 output rows (M=124 partitions, N=256) which are
directly contiguous NHWC rows in DRAM. The per-sample bias
(bias * Berr[b]) is added during the PSUM->SBUF move by the DVE.
"""

import numpy as np

B, H, Wd, CIN, COUT, KH, KW = 64, 64, 64, 128, 256, 3, 3
HO, WO = H - KH + 1, Wd - KW + 1  # 62, 62
NCORES = 8
S = B // NCORES  # samples per core
ROWS_PER_CHUNK = 2
NCHUNK = HO // ROWS_PER_CHUNK  # 31
M = ROWS_PER_CHUNK * WO  # 124

PAD = 64  # X tile free-dim pad: last chunk's kh=2/kw>0 taps read past H*W

TRACE = False  # set by test harness to capture an NTFF profile
LAST_RESULTS = None  # BassKernelResults of the most recent run (for profiling)

_prog_cache = None


def _build_program():
    import concourse.mybir as mybir
    from concourse import bacc
    from concourse.tile import TileContext

    f32 = mybir.dt.float32
    f32r = mybir.dt.float32r

    # Bacc (not plain Bass): its compile() runs generate_event_semaphores,
    # which splits multi-sem waits into EventSemaphore chains — walrus
    # codegen rejects instructions carrying more than ~2 sync waits.
    nc = bacc.Bacc()

    # X_t is declared float32r (same bytes as f32): walrus requires data
    # consumed by an f32r matmul to be produced as f32r along the whole chain.
    # The free dim carries a host-zeroed PAD so the last chunk's kh=2 taps
    # can read one full 128-wide stationary without going out of bounds.
    X_t = nc.declare_dram_parameter(
        "X_t", [S, CIN, H * Wd + PAD], f32r, isOutput=False
    )
    W_p = nc.declare_dram_parameter("W", [KH * KW * CIN, COUT], f32, isOutput=False)
    bias_p = nc.declare_dram_parameter("bias", [COUT], f32, isOutput=False)
    Werr_p = nc.declare_dram_parameter(
        "Werr", [S, KH * KW * CIN, COUT], f32, isOutput=False
    )
    Berr_p = nc.declare_dram_parameter("Berr", [S, COUT], f32, isOutput=False)
    # output rows are stored 64 wide (2 dead columns) so each chunk is one
    # full-partition DMA; the host strips the padding
    OUT = nc.declare_dram_parameter("OUT", [S, HO * Wd, COUT], f32, isOutput=True)

    TAPF = KH * KW * COUT  # 2304 free elems: tap t occupies cols [t*COUT, (t+1)*COUT)

    with TileContext(nc) as tc:
        with (
            tc.tile_pool(name="const", bufs=1) as cpool,
            tc.tile_pool(name="xp", bufs=2) as xpool,
            tc.tile_pool(name="wep", bufs=2) as wepool,
            tc.tile_pool(name="mwp", bufs=2) as mwpool,
            tc.tile_pool(name="bbp", bufs=2) as bbpool,
            tc.tile_pool(name="outp", bufs=6) as opool,
            tc.tile_pool(name="ps", bufs=8, space="PSUM") as pspool,
        ):
            # W taps, resident all run: [cin, (t cout)]
            W_sb = cpool.tile([CIN, TAPF], f32)
            nc.sync.dma_start(
                out=W_sb[:].rearrange("p (t j) -> p t j", j=COUT),
                in_=W_p[:].rearrange("(t p) j -> p t j", p=CIN),
            )
            # bias broadcast to all partitions: [128, COUT]
            bias_bc = cpool.tile([128, COUT], f32)
            nc.gpsimd.dma_start(out=bias_bc, in_=bias_p[:].partition_broadcast(128))

            for s in range(S):
                # X load split into row-pieces so chunk 0's matmuls only wait
                # on the first piece, not the whole 2 MiB transfer
                X_sb = xpool.tile([CIN, H * Wd + PAD], f32r)
                XPIECE = 16 * Wd  # 16 rows
                for off in range(0, H * Wd + PAD, XPIECE):
                    end = min(off + XPIECE, H * Wd + PAD)
                    nc.sync.dma_start(
                        out=X_sb[:, off:end], in_=X_t[s, :, off:end]
                    )

                # Werr load + memW mul split into 3 tap-groups: the first
                # matmuls need only taps 0-2, and the DVE work interleaves
                # with the per-chunk adds instead of a 2.4us monolith
                Werr_sb = wepool.tile([CIN, TAPF], f32)
                memW = mwpool.tile([CIN, TAPF], f32r)
                GRP = 3 * COUT
                for g in range(3):
                    lo, hi = g * GRP, (g + 1) * GRP
                    nc.sync.dma_start(
                        out=Werr_sb[:, lo:hi].rearrange(
                            "p (t j) -> p t j", j=COUT
                        ),
                        in_=Werr_p[
                            s, g * 3 * CIN : (g + 1) * 3 * CIN, :
                        ].rearrange("(t p) j -> p t j", p=CIN),
                    )
                    nc.vector.tensor_mul(
                        memW[:, lo:hi], W_sb[:, lo:hi], Werr_sb[:, lo:hi]
                    )

                berr_bc = bbpool.tile([128, COUT], f32)
                nc.gpsimd.dma_start(
                    out=berr_bc, in_=Berr_p[s].partition_broadcast(128)
                )
                membias = bbpool.tile([128, COUT], f32)
                nc.vector.tensor_mul(membias, bias_bc, berr_bc)

                # Each chunk covers 2 output rows as 128 PSUM partitions in
                # 64-wide row coordinates: partition m = (ho - 2c)*64 + wo,
                # wo in [0,64) with wo in {62,63} dead. The tap (kh,kw)
                # stationary is then the single contiguous X slab starting at
                # (2c+kh)*64 + kw — one free dim, as walrus requires.
                for c in range(NCHUNK):
                    ps = pspool.tile([128, COUT], f32)
                    mm = 0
                    for kh in range(KH):
                        for kw in range(KW):
                            t = kh * KW + kw
                            base = (ROWS_PER_CHUNK * c + kh) * Wd + kw
                            lhsT = X_sb[:, base : base + 128]
                            rhs = memW[:, t * COUT : (t + 1) * COUT]  # [128, 256]
                            nc.tensor.matmul(
                                ps[:],
                                lhsT,
                                rhs,
                                start=(mm == 0),
                                stop=(mm == KH * KW - 1),
                            )
                            mm += 1
                    o_sb = opool.tile([128, COUT], f32)
                    nc.vector.tensor_add(o_sb, ps, membias)
                    # out stores ride ACT's HWDGE so SP's queue clocks (wide
                    # X/Werr loads) and these narrow stores stay independent
                    nc.scalar.dma_start(
                        out=OUT[s, 128 * c : 128 * (c + 1), :], in_=o_sb
                    )

    nc.compile()
    return nc


def _get_program():
    global _prog_cache
    if _prog_cache is None:
        _prog_cache = _build_program()
    return _prog_cache


def kernel(X, W, bias, Werr, Berr):
    global LAST_RESULTS
    from concourse.bass_utils import run_bass_kernel_spmd

    X = np.asarray(X, dtype=np.float32)
    W = np.asarray(W, dtype=np.float32)
    bias = np.asarray(bias, dtype=np.float32)
    Werr = np.asarray(Werr, dtype=np.float32)
    Berr = np.asarray(Berr, dtype=np.float32)

    # host-side layout prep (part of sharding): Cin onto partitions, zero pad
    X_t = np.zeros((B, CIN, H * Wd + PAD), np.float32)
    X_t[:, :, : H * Wd] = X.transpose(0, 3, 1, 2).reshape(B, CIN, H * Wd)
    W2 = np.ascontiguousarray(W.reshape(KH * KW * CIN, COUT))
    Werr2 = np.ascontiguousarray(Werr.reshape(B, KH * KW * CIN, COUT))
    Berr2 = np.ascontiguousarray(Berr)

    nc = _get_program()
    in_maps = []
    for core in range(NCORES):
        sl = slice(core * S, (core + 1) * S)
        in_maps.append(
            {
                "X_t": X_t[sl],
                "W": W2,
                "bias": bias,
                "Werr": Werr2[sl],
                "Berr": Berr2[sl],
            }
        )

    res = run_bass_kernel_spmd(
        nc, in_maps, core_ids=list(range(NCORES)), trace=TRACE
    )
    LAST_RESULTS = res
    out = np.concatenate([r["OUT"] for r in res.results], axis=0)
    # rows are stored 64 wide on device; strip the 2 dead columns
    return np.ascontiguousarray(
        out.reshape(B, HO, Wd, COUT)[:, :, :WO, :]
    )
